# revision 58
# baseline (speedup 1.0000x reference)
"""Trainium2 Bass kernel for topk_masking (nn_CustomModule_8065948582484).

Reference semantics (per batch b):
  idx[b,f] = argmax(score[b,f,:196])                 (first index on ties)
  mask grows from a fixed prior region on a 14x14 grid; at frame f the
  argmax cell is added iff it is 4-adjacent to the current mask.
  out = [ones(B,1), masks frame-major] -> [B, 1+16*196] fp32.

Strategy (pure batch data-parallel across 8 cores, 2048 batches/core,
16 tiles of 128 batches on SBUF partitions, processed in groups of 8).
Work is spread across Pool/DVE/ScalarE so every engine stays under the
~143us serial DMA floor (25.7MB in + 25.7MB out per core @360GB/s):
  1. guard-free prefix-max scan on Pool (gpsimd):
     state = max(state + d0[t], score[t]) with d0 = -2e30 at frame
     starts, 0 elsewhere -- score DMA stays fully contiguous.
  2. idx = popcount(prefix_max < max): frames 0..K-1 per tile via DVE
     scalar_tensor_tensor(is_lt, accum_out), frames K..15 via ScalarE
     sign+accumulator. Exact incl. first-index tie semantics.
  3. r via popcount over row-end prefix-max slots; c = idx - 14r.
  4. per-frame "added" recurrence on a 16-node adjacency graph, batched
     across 8-tile groups; adjacency via dv^2 in {1,256} (Pool).
  5. masks: fused one-hot+running-max (scalar_tensor_tensor is_equal/max)
     writing fp32 straight into the output tile -- no convert pass.
  6. input DMA issued from the idle sync queue; output DMA issued from
     DVE (the producer) so no sequencer ever blocks on unmet deps.
"""

import sys

import numpy as np

for _p in ("/opt/trn_rl_repo",):
    if _p not in sys.path:
        sys.path.insert(0, _p)

from concourse import bacc, mybir, tile  # noqa: E402
from concourse.bass_utils import run_bass_kernel_spmd  # noqa: E402

B, F, P = 16384, 16, 196
N = 14  # grid side
NCORES = 8
BLOC = B // NCORES  # 2048
NT = BLOC // 128  # 16 tiles per core
GROUPS = [4, 4, 5, 3]  # tiles per phase-2 group (sum == NT)
NG = len(GROUPS)
GMAX = max(GROUPS)
# frames per tile whose popcount runs on DVE (rest on ScalarE), by tile.
# Front tiles lean on DVE (idle while scans stream in); back tiles lean on
# ScalarE (free once its early popcounts are done) so DVE can push chains.
KDS = [5] * 8 + [0] * 8
SCAN_SPLIT = 2  # sub-scans (and input sub-DMAs) per tile: latency knob

PHASE_CB = None  # optional profiling hook: called with a label at phase starts

ALU = mybir.AluOpType
AX = mybir.AxisListType
F32 = mybir.dt.float32
BF16 = mybir.dt.bfloat16
ACT = mybir.ActivationFunctionType
BIG = 1e30


def build_nc():
    nc = bacc.Bacc(trn_type="TRN2", target_bir_lowering=False)
    score_d = nc.declare_dram_parameter("score", [BLOC, F, P], F32, isOutput=False)
    out_d = nc.declare_dram_parameter("out", [BLOC, 1 + F * P], F32, isOutput=True)

    with tile.TileContext(nc) as tc:
        with (
            tc.tile_pool(name="consts", bufs=1) as cpool,
            tc.tile_pool(name="big", bufs=2) as bpool,
            tc.tile_pool(name="grp", bufs=2) as gpool,
        ):
            # ---- constants ----
            # iota from 1: the chain compares (idx+1)*added against it, so
            # added=0 (product 0) never matches -- no separate -1 needed.
            iotap = cpool.tile([128, P], BF16, name="iotap")
            nc.gpsimd.iota(
                iotap[:],
                pattern=[[1, P]],
                base=1,
                channel_multiplier=0,
                allow_small_or_imprecise_dtypes=True,
            )
            prior = cpool.tile([128, P], BF16, name="prior")
            nc.vector.memset(prior[:], 0.0)
            priorv = prior.rearrange("q (r c) -> q r c", r=N)
            nc.vector.memset(priorv[:, 4:14, 2:12], 1.0)
            # scan reset pattern: state = max(state + d0[t], score[t])
            d0 = cpool.tile([128, F * P], BF16, name="d0")
            nc.vector.memset(d0[:], 0.0)
            d0v = d0.rearrange("q (f p) -> q f p", f=F)
            nc.vector.memset(d0v[:, :, 0:1], -2 * BIG)
            zcol = cpool.tile([128, 1], BF16, name="zcol")
            nc.vector.memset(zcol[:], 0.0)
            # row thresholds 14, 28, ..., 182: r = sum_k [idx >= 14k]
            thr = cpool.tile([128, N - 1], BF16, name="thr")
            nc.gpsimd.iota(
                thr[:],
                pattern=[[N, N - 1]],
                base=N,
                channel_multiplier=0,
                allow_small_or_imprecise_dtypes=True,
            )

            # per-group accumulators (bufs=2: adjacent groups overlap)
            idxa_g = {}
            G0 = [sum(GROUPS[:g]) for g in range(NG)]  # first tile of group g

            def alloc_group(g):
                idxa_g[g] = gpool.tile([128, F, GMAX], F32, tag="idxa", name="idxa")

            idxm_g = [None] * NG

            def phase_a(g, j):
                """load / scan / popcounts for tile (g, j)."""
                PHASE_CB and PHASE_CB(f"A{g}.{j}")
                idxa = idxa_g[g]
                r0 = (G0[g] + j) * 128
                sc = bpool.tile([128, F * P], F32, tag="sc", name="sc", bufs=3)
                run = bpool.tile([128, F * P], F32, tag="run", name="run", bufs=3)
                scd = score_d.rearrange("b f p -> b (f p)")
                fs = F // SCAN_SPLIT
                for s in range(SCAN_SPLIT):
                    seg = slice(s * fs * P, (s + 1) * fs * P)
                    nc.sync.dma_start(
                        out=sc[:, seg], in_=scd[r0 : r0 + 128, seg]
                    )
                    # d0's -2e30 reset at each frame start makes every
                    # sub-scan independent (initial 0 is below any reset)
                    nc.vector.tensor_tensor_scan(
                        run[:, seg], d0[:, seg], sc[:, seg], 0.0, ALU.add, ALU.max
                    )
                runv = run.rearrange("q (f p) -> q f p", f=F)
                # idx = #positions with prefix-max below the frame max
                zb = zcol.broadcast_to([128, P])
                kd = KDS[G0[g] + j]
                for f in range(F):
                    if f < kd:
                        # DVE: (run < m) summed by the STT accumulator
                        jk = gpool.tile([128, P], BF16, tag="jkd", name="jkd", bufs=2)
                        nc.vector.scalar_tensor_tensor(
                            jk[:],
                            runv[:, f, 0:P],
                            runv[:, f, P - 1 : P],
                            zb,
                            ALU.is_lt,
                            ALU.max,
                            accum_out=idxa[:, f, j : j + 1],
                        )
                    else:
                        # ScalarE: sign(m - run) in {1,0}, accumulated
                        jk = gpool.tile([128, P], BF16, tag="jka", name="jka", bufs=2)
                        nc.scalar.activation(
                            jk[:],
                            runv[:, f, 0:P],
                            ACT.Sign,
                            bias=runv[:, f, P - 1 : P],
                            scale=-1.0,
                            accum_out=idxa[:, f, j : j + 1],
                        )
            def phase_b(g):
                """batched small compute for the whole group -> idxm."""
                PHASE_CB and PHASE_CB(f"B{g}")
                gs = GROUPS[g]
                idxa = idxa_g[g][:, :, :gs]
                # r = #row-thresholds idx reaches; c = idx - 14 r
                # (pitch-16 cell id v = 16 r + c: adjacency <=> |dv| in {1,16})
                ucmp = gpool.tile(
                    [128, F, GMAX, N - 1], BF16, tag="ucmp", name="ucmp"
                )[:, :, :gs, :]
                nc.vector.tensor_tensor(
                    ucmp,
                    idxa.unsqueeze(3).broadcast_to([128, F, gs, N - 1]),
                    thr.unsqueeze(1).unsqueeze(1).broadcast_to([128, F, gs, N - 1]),
                    ALU.is_ge,
                )
                rr = gpool.tile([128, F, GMAX], F32, tag="rr", name="rr")[:, :, :gs]
                nc.vector.tensor_reduce(rr, ucmp, axis=AX.X, op=ALU.add)
                ccf = gpool.tile([128, F, GMAX], F32, tag="cc", name="cc")
                cc = ccf[:, :, :gs]
                nc.vector.scalar_tensor_tensor(
                    cc, rr, -float(N), idxa, ALU.mult, ALU.add
                )
                vv = gpool.tile([128, F, GMAX], F32, tag="vv", name="vv")[:, :, :gs]
                nc.vector.scalar_tensor_tensor(vv, rr, 16.0, cc, ALU.mult, ALU.add)
                vb = gpool.tile([128, F, GMAX], BF16, tag="vb", name="vb")[:, :, :gs]
                nc.vector.tensor_copy(vb, vv)

                # adjacency gg[e,f,t] = (dv^2 == 1) + (dv^2 == 256)
                dv = gpool.tile([128, F, F, GMAX], BF16, tag="dv", name="dv")[
                    :, :, :, :gs
                ]
                nc.vector.tensor_tensor(
                    dv,
                    vb.unsqueeze(2).broadcast_to([128, F, F, gs]),
                    vb.unsqueeze(1).broadcast_to([128, F, F, gs]),
                    ALU.subtract,
                )
                sq = gpool.tile([128, F, F, GMAX], BF16, tag="sq", name="sq")[
                    :, :, :, :gs
                ]
                nc.vector.tensor_tensor(sq, dv, dv, ALU.mult)
                g1 = gpool.tile([128, F, F, GMAX], BF16, tag="g1", name="g1")[
                    :, :, :, :gs
                ]
                nc.vector.tensor_scalar(g1, sq, 1.0, None, ALU.is_equal)
                g16 = gpool.tile([128, F, F, GMAX], BF16, tag="g16", name="g16")[
                    :, :, :, :gs
                ]
                nc.vector.tensor_scalar(g16, sq, 256.0, None, ALU.is_equal)
                gg = gpool.tile([128, F, F, GMAX], BF16, tag="gg", name="gg")[
                    :, :, :, :gs
                ]
                nc.vector.tensor_tensor(gg, g1, g16, ALU.add)

                # A = (r>=3 & 2<=c<=11) | (r>=4 & 1<=c<=12)
                u3 = gpool.tile([128, F, GMAX], BF16, tag="u3", name="u3")[:, :, :gs]
                nc.vector.tensor_scalar(u3, rr, 3.0, None, ALU.is_ge)
                u4 = gpool.tile([128, F, GMAX], BF16, tag="u4", name="u4")[:, :, :gs]
                nc.vector.tensor_scalar(u4, rr, 4.0, None, ALU.is_ge)
                cm2 = gpool.tile([128, F, GMAX], F32, tag="cm2", name="cm2")[:, :, :gs]
                nc.vector.tensor_scalar(cm2, cc, 2.0, None, ALU.subtract)
                q1 = gpool.tile([128, F, GMAX], F32, tag="q1", name="q1")[:, :, :gs]
                nc.vector.scalar_tensor_tensor(q1, cc, -11.0, cm2, ALU.add, ALU.mult)
                b1 = gpool.tile([128, F, GMAX], BF16, tag="b1", name="b1")[:, :, :gs]
                nc.vector.tensor_scalar(b1, q1, 0.0, None, ALU.is_le)
                cm1 = gpool.tile([128, F, GMAX], F32, tag="cm1", name="cm1")[:, :, :gs]
                nc.vector.tensor_scalar(cm1, cc, 1.0, None, ALU.subtract)
                q2 = gpool.tile([128, F, GMAX], F32, tag="q2", name="q2")[:, :, :gs]
                nc.vector.scalar_tensor_tensor(q2, cc, -12.0, cm1, ALU.add, ALU.mult)
                b2 = gpool.tile([128, F, GMAX], BF16, tag="b2", name="b2")[:, :, :gs]
                nc.vector.tensor_scalar(b2, q2, 0.0, None, ALU.is_le)
                t1 = gpool.tile([128, F, GMAX], BF16, tag="t1", name="t1")[:, :, :gs]
                nc.vector.tensor_tensor(t1, u3, b1, ALU.logical_and)
                t2 = gpool.tile([128, F, GMAX], BF16, tag="t2", name="t2")[:, :, :gs]
                nc.vector.tensor_tensor(t2, u4, b2, ALU.logical_and)
                aa = gpool.tile([128, F, GMAX], F32, tag="aa", name="aa")[:, :, :gs]
                nc.vector.tensor_tensor(aa, t1, t2, ALU.logical_or)

                # sequential added-recurrence, batched over the group:
                # added[f] = max(A[f], max_e added[e]*G[e,f])
                added = gpool.tile([128, F, GMAX], BF16, tag="added", name="added")[
                    :, :, :gs
                ]
                nc.vector.memset(added, 0.0)
                t16 = gpool.tile([128, F, GMAX], BF16, tag="t16", name="t16")[
                    :, :, :gs
                ]
                mx = gpool.tile([128, GMAX], F32, tag="mx", name="mx")[:, :gs]
                for f in range(F):
                    nc.vector.tensor_tensor(t16, added, gg[:, :, f, :], ALU.mult)
                    t16v = t16.rearrange("q e t -> q t e")
                    nc.vector.tensor_reduce(mx, t16v, axis=AX.X, op=ALU.max)
                    nc.vector.tensor_tensor(
                        added[:, f, :], mx, aa[:, f, :], ALU.max
                    )

                # idxm[f] = added[f] ? idx[f]+1 : 0   (iotap runs 1..P)
                idxm = gpool.tile([128, F, GMAX], F32, tag="idxm", name="idxm")
                nc.vector.scalar_tensor_tensor(
                    idxm[:, :, :gs], idxa, 1.0, added, ALU.add, ALU.mult
                )
                idxm_g[g] = idxm

            out_tiles = {}

            HALF = 1 + (F // 2) * P  # ones column + frames 0..7

            def chain(g, j):
                """fused one-hot+cummax mask build, bf16; the lower half of
                the output ships as soon as frame 7's step completes."""
                PHASE_CB and PHASE_CB(f"C{g}.{j}")
                idxm = idxm_g[g]
                r0 = (G0[g] + j) * 128
                out_t = bpool.tile(
                    [128, 1 + F * P], BF16, tag="out", name="out_t", bufs=5
                )
                nc.gpsimd.memset(out_t[:, 0:1], 1.0)
                for f in range(F):
                    prev = (
                        prior[:] if f == 0 else out_t[:, 1 + (f - 1) * P : 1 + f * P]
                    )
                    # out = max(prev, iota == idxm[f])
                    nc.vector.scalar_tensor_tensor(
                        out_t[:, 1 + f * P : 1 + (f + 1) * P],
                        iotap[:],
                        idxm[:, f, j : j + 1],
                        prev,
                        ALU.is_equal,
                        ALU.max,
                    )
                    if f == F // 2 - 1:
                        # software-DGE DMA on the otherwise-idle Pool queue;
                        # it also casts bf16 -> fp32 in flight (gpsimd-only)
                        nc.gpsimd.dma_start(
                            out=out_d[r0 : r0 + 128, 0:HALF],
                            in_=out_t[:, 0:HALF],
                        )
                out_tiles[(g, j)] = out_t

            def out_dma(g, j):
                PHASE_CB and PHASE_CB(f"O{g}.{j}")
                r0 = (G0[g] + j) * 128
                nc.gpsimd.dma_start(
                    out=out_d[r0 : r0 + 128, HALF:],
                    in_=out_tiles.pop((g, j))[:, HALF:],
                )

            # software pipeline. Emission order per group: phase A, then
            # phase B (so B never queues behind the previous group's mask
            # chains on DVE), then the previous group's chains; stores issue
            # on Pool's software-DGE queue, which hosts nothing else.
            alloc_group(0)
            for j in range(GROUPS[0]):
                phase_a(0, j)
            phase_b(0)
            for g in range(1, NG):
                alloc_group(g)
                for j in range(GROUPS[g]):
                    phase_a(g, j)
                phase_b(g)
                for j in range(GROUPS[g - 1]):
                    chain(g - 1, j)
                    out_dma(g - 1, j)
            for j in range(GROUPS[NG - 1]):
                chain(NG - 1, j)
                out_dma(NG - 1, j)

    nc.compile()
    return nc


_nc = None


def _get_nc():
    global _nc
    if _nc is None:
        _nc = build_nc()
    return _nc


def kernel(score, topn=196):
    score = np.ascontiguousarray(np.asarray(score, dtype=np.float32)).reshape(B, F, P)
    nc = _get_nc()
    in_maps = [
        {"score": score[i * BLOC : (i + 1) * BLOC]} for i in range(NCORES)
    ]
    res = run_bass_kernel_spmd(nc, in_maps, list(range(NCORES)))
    out = np.concatenate([res.results[i]["out"] for i in range(NCORES)], axis=0)
    return out
